# revision 1
# baseline (speedup 1.0000x reference)
"""PosGCN Trainium2 kernel: out = x + relu(segment_sum((x@W)[edge_src], edge_dst) + b).

Distribution: 1D node partition across 8 NeuronCores. Core c owns dst nodes
[c*12500, (c+1)*12500) and the edges incident to them (partitioned by dst).
W/b replicated; x replicated into every core's HBM so cross-partition source
rows are gathered locally (no collectives).

Key algebraic rewrite: aggregation is linear, so
    segment_sum((x@W)[src]) == segment_sum(x[src]) @ W
We aggregate raw x features per dst first (the memory-bound part), then apply
the dense transform to the 12500 aggregated rows per core.

Gather: the production InstDMAGatherAnt (nc.gpsimd.dma_gather) fetches up to
thousands of 512B rows per instruction. Its indices are int16, so x rows are
addressed quadrant-relative (4 quadrants of 25000 rows); each core's edges are
sorted by (dst-chunk, src-quadrant) and each (chunk, quadrant) run is padded
to a multiple of 128 edges (pad: src=quadrant row 0, dst sentinel -> one-hot
row of zeros -> contributes nothing).

Precision: x is split into bf16 hi + bf16 lo (x ~= hi + lo, ~16 mantissa bits)
packed per row as [hi(128) | lo(128)] so a gathered row stays one 512B unit.
The one-hot segment-sum matmuls run in bf16 (exact 0/1 one-hot) accumulating
in f32 PSUM -> ~1e-5 relative error, at full PE rate.
"""

import sys
from contextlib import ExitStack

import numpy as np

sys.path.insert(0, "/opt/trn_rl_repo")

import ml_dtypes

import concourse.bass as bass
import concourse.tile as tile
from concourse import bacc, mybir
from concourse.bass_utils import run_bass_kernel_spmd

P = 128
N_NODES = 100000
N_EDGES = 1600000
D = 128
N_CORES = 8
QN = 25000  # quadrant size (int16 gather indices must stay < 32768)
NQ = 4

f32 = mybir.dt.float32
bf16 = mybir.dt.bfloat16
i16 = mybir.dt.int16

# test.py can read results metadata from here after a run
last_results = None


def _cdiv(a, b):
    return (a + b - 1) // b


def _build_tables(edge_src, edge_dst, n_nodes, n_cores, qn):
    """Sort each core's edges by (dst-chunk, src-quadrant); build gather runs.

    Returns (runs, tile_base, T, per_core) where:
      runs: list of (chunk k, quadrant q, tile_start ts, n_tiles nt) — one
            dma_gather instruction each; identical across cores (SPMD).
      tile_base[k]: first global tile of chunk k.
      per_core[c] = (idx16_tbl [128, T*8] int16, dstr_tbl [128, T] f32).
    Edge-tile t (128 edges) occupies flat slots [t*128, (t+1)*128); lane p of
    tile t is flat slot t*128+p (matches dma_gather's dst[i%128, i//128]).
    """
    NS = n_nodes // n_cores
    CH = _cdiv(NS, P)
    nq = _cdiv(n_nodes, qn)

    order = np.argsort(edge_dst, kind="stable")
    ss = np.asarray(edge_src)[order].astype(np.int64)
    ds = np.asarray(edge_dst)[order].astype(np.int64)
    bounds = np.searchsorted(ds, np.arange(n_cores + 1) * NS)

    counts = np.zeros((n_cores, CH, nq), np.int64)
    segs = []
    for c in range(n_cores):
        b0, b1 = bounds[c], bounds[c + 1]
        src_c = ss[b0:b1]
        ldst = ds[b0:b1] - c * NS
        quad = src_c // qn
        o2 = np.lexsort((quad, ldst // P))
        src_c, ldst, quad = src_c[o2], ldst[o2], quad[o2]
        np.add.at(counts[c], (ldst // P, quad), 1)
        segs.append((src_c, ldst))

    tiles_kq = _cdiv(counts.max(axis=0), P)  # [CH, nq]
    for k in range(CH):
        if tiles_kq[k].sum() == 0:
            tiles_kq[k][0] = 1  # keep >=1 tile so the chunk's PSUM is written

    runs = []
    tile_base = np.zeros(CH + 1, np.int64)
    t = 0
    for k in range(CH):
        tile_base[k] = t
        for q in range(nq):
            nt = int(tiles_kq[k][q])
            if nt:
                runs.append((k, q, t, nt))
                t += nt
    tile_base[CH] = t
    T = t

    per_core = []
    for c in range(n_cores):
        src_c, ldst = segs[c]
        # per-(k,q) offsets into the sorted edge arrays
        offs = np.zeros((CH, nq + 1), np.int64)
        offs[:, 1:] = np.cumsum(counts[c], axis=1)
        row_off = np.concatenate([[0], np.cumsum(counts[c].sum(axis=1))])
        flat_src = np.zeros(T * P, np.int16)
        flat_dstr = np.full(T * P, float(P), np.float32)
        for k, q, ts, nt in runs:
            n = int(counts[c][k][q])
            if n == 0:
                continue
            s = int(row_off[k] + offs[k][q])
            slot = ts * P
            flat_src[slot : slot + n] = (src_c[s : s + n] - q * qn).astype(np.int16)
            flat_dstr[slot : slot + n] = (ldst[s : s + n] % P).astype(np.float32)
        # dma_gather wrapped-index layout: idxs[p, s] = flat[s*16 + p], p<16,
        # replicated over the other 112 partitions
        idx16 = np.tile(flat_src.reshape(T * 8, 16).T, (8, 1))
        per_core.append(
            (
                np.ascontiguousarray(idx16),
                np.ascontiguousarray(flat_dstr.reshape(T, P).T),
            )
        )
    return runs, tile_base, T, per_core


def _build_program(n_nodes, NS, runs, tile_base, T, qn, reps=1, skip=()):
    """Emit the SPMD Bass program for one core (identical across cores).

    reps > 1 replicates the whole body (for slope-based HW timing; dispatch
    overhead cancels between two rep counts).
    """
    CH = len(tile_base) - 1
    last_w = NS - (CH - 1) * P

    nc = bacc.Bacc(
        "TRN2",
        target_bir_lowering=False,
        debug=False,
        num_devices=N_CORES,
        num_swdge_queues=4,
    )

    xp = nc.dram_tensor("xp", [n_nodes, 2 * D], bf16, kind="ExternalInput")
    idx16 = nc.dram_tensor("idx16", [P, T * 8], i16, kind="ExternalInput")
    dstr = nc.dram_tensor("dstr", [P, T], f32, kind="ExternalInput")
    xrt = nc.dram_tensor("xrt", [P, NS], f32, kind="ExternalInput")
    w = nc.dram_tensor("w", [P, D], f32, kind="ExternalInput")
    bcol = nc.dram_tensor("b", [P, 1], f32, kind="ExternalInput")
    iota = nc.dram_tensor("iota", [P, P], f32, kind="ExternalInput")
    outT = nc.dram_tensor("outT", [P, NS], f32, kind="ExternalOutput")

    # tile t -> (its gather run, index within run)
    tile_run = {}
    for ri, (k, q, ts, nt) in enumerate(runs):
        for j in range(nt):
            tile_run[ts + j] = (ri, j)

    with tile.TileContext(nc) as tc, ExitStack() as ctx:
        const = ctx.enter_context(tc.tile_pool(name="const", bufs=1))
        tbl = ctx.enter_context(tc.tile_pool(name="tbl", bufs=1))
        gpool = ctx.enter_context(tc.tile_pool(name="gather", bufs=4))
        opool = ctx.enter_context(tc.tile_pool(name="onehot", bufs=6))
        apsum = ctx.enter_context(tc.tile_pool(name="apsum", bufs=2, space="PSUM"))
        opsum = ctx.enter_context(tc.tile_pool(name="opsum", bufs=2, space="PSUM"))
        asb = ctx.enter_context(tc.tile_pool(name="asb", bufs=2))
        osb = ctx.enter_context(tc.tile_pool(name="osb", bufs=3))
        xrp = ctx.enter_context(tc.tile_pool(name="xrp", bufs=3))

        def emit_body():
            w_sb = const.tile([P, D], f32)
            nc.sync.dma_start(out=w_sb[:], in_=w[:])
            b_sb = const.tile([P, 1], f32)
            nc.sync.dma_start(out=b_sb[:], in_=bcol[:])
            io_sb = const.tile([P, P], f32)
            nc.sync.dma_start(out=io_sb[:], in_=iota[:])
            idx_sb = tbl.tile([P, T * 8], i16)
            nc.sync.dma_start(out=idx_sb[:], in_=idx16[:])
            dstr_sb = tbl.tile([P, T], f32)
            nc.sync.dma_start(out=dstr_sb[:], in_=dstr[:])

            gbufs = {}
            if "onehot" in skip:
                ohc = const.tile([P, P], bf16, tag="ohc")
                nc.vector.memset(ohc[:], 0.0)
            if "gather" in skip:
                dummy = const.tile([P, 1, 2 * D], bf16, tag="dummy")
                nc.vector.memset(dummy[:], 0.0)

            def gather_run(ri):
                k, q, ts, nt = runs[ri]
                if "gather" in skip:
                    gbufs[ri] = dummy
                    return
                gdim = D if "half" in skip else 2 * D
                gb = gpool.tile([P, nt, gdim], bf16, tag="gb")
                halves = 2 if "split2" in skip else 1
                qnum = 0 if "queue0" in skip else ri % 4
                bounds_t = [0, nt] if halves == 1 else [0, (nt + 1) // 2, nt]
                for h in range(halves):
                    a, bnd = bounds_t[h], bounds_t[h + 1]
                    if bnd == a:
                        continue
                    if "half" in skip:
                        nc.gpsimd.dma_gather(
                            gb[:, a:bnd, :],
                            xp[q * qn : (q + 1) * qn, 0:D],
                            idx_sb[:, (ts + a) * 8 : (ts + bnd) * 8],
                            (bnd - a) * P,
                            (bnd - a) * P,
                            D,
                            elem_step=2 * D,
                            queue_num=qnum,
                        )
                    else:
                        nc.gpsimd.dma_gather(
                            gb[:, a:bnd, :],
                            xp[q * qn : (q + 1) * qn, :],
                            idx_sb[:, (ts + a) * 8 : (ts + bnd) * 8],
                            (bnd - a) * P,
                            (bnd - a) * P,
                            2 * D,
                            elem_step=2 * D,
                            queue_num=qnum,
                        )
                gbufs[ri] = gb

            for k in range(CH):
                t0, t1 = int(tile_base[k]), int(tile_base[k + 1])
                nt_k = t1 - t0
                psum = apsum.tile([P, P], f32)
                if "mm" in skip:
                    nc.vector.memset(psum[:], 0.0)
                for j in range(nt_k):
                    t = t0 + j
                    ri, jr = tile_run[t]
                    if ri not in gbufs:
                        gather_run(ri)
                    gb = gbufs[ri]
                    oh = ohc if "onehot" in skip else opool.tile([P, P], bf16)
                    if "onehot" not in skip:
                        nc.vector.tensor_tensor(
                            out=oh[:],
                            in0=dstr_sb[:, t : t + 1].to_broadcast([P, P]),
                            in1=io_sb[:],
                            op=mybir.AluOpType.is_equal,
                        )
                    if "mm" not in skip:
                        jg = 0 if "gather" in skip else jr
                        hi = gb[:, jg, 0:D]
                        lo = hi if "half" in skip else gb[:, jg, D : 2 * D]
                        nc.tensor.matmul(
                            out=psum[:], lhsT=hi, rhs=oh[:], start=(j == 0), stop=False
                        )
                        nc.tensor.matmul(
                            out=psum[:],
                            lhsT=lo,
                            rhs=oh[:],
                            start=False,
                            stop=(j == nt_k - 1),
                        )

                # aggT[f, d] (PSUM) -> out2T[g, d] = sum_f W[f,g] * aggT[f,d]
                aggT = asb.tile([P, P], f32)
                nc.vector.tensor_copy(out=aggT[:], in_=psum[:])
                po = opsum.tile([P, P], f32)
                nc.tensor.matmul(out=po[:], lhsT=w_sb[:], rhs=aggT[:], start=True, stop=True)
                ot = osb.tile([P, P], f32)
                nc.scalar.activation(
                    out=ot[:],
                    in_=po[:],
                    func=mybir.ActivationFunctionType.Relu,
                    bias=b_sb[:],
                )
                wk = P if k < CH - 1 else last_w
                xr = xrp.tile([P, P], f32)
                nc.sync.dma_start(out=xr[:, :wk], in_=xrt[:, k * P : k * P + wk])
                nc.vector.tensor_add(out=ot[:, :wk], in0=ot[:, :wk], in1=xr[:, :wk])
                nc.sync.dma_start(out=outT[:, k * P : k * P + wk], in_=ot[:, :wk])

        for _rep in range(reps):
            emit_body()

    nc.compile()
    return nc


def _make_in_maps(x, edge_src, edge_dst, W, b, n_nodes, n_cores, qn):
    NS = n_nodes // n_cores
    xf = np.ascontiguousarray(np.asarray(x, dtype=np.float32))
    x_hi = xf.astype(ml_dtypes.bfloat16)
    x_lo = (xf - x_hi.astype(np.float32)).astype(ml_dtypes.bfloat16)
    xp = np.ascontiguousarray(np.concatenate([x_hi, x_lo], axis=1))

    runs, tile_base, T, per_core = _build_tables(
        np.asarray(edge_src), np.asarray(edge_dst), n_nodes, n_cores, qn
    )

    w_np = np.ascontiguousarray(np.asarray(W, dtype=np.float32))
    b_np = np.ascontiguousarray(np.asarray(b, dtype=np.float32).reshape(P, 1))
    iota_np = np.ascontiguousarray(
        np.broadcast_to(np.arange(P, dtype=np.float32), (P, P))
    )

    in_maps = []
    for c in range(n_cores):
        idx16_tbl, dstr_tbl = per_core[c]
        in_maps.append(
            {
                "xp": xp,
                "idx16": idx16_tbl,
                "dstr": dstr_tbl,
                "xrt": np.ascontiguousarray(xf[c * NS : (c + 1) * NS].T),
                "w": w_np,
                "b": b_np,
                "iota": iota_np,
            }
        )
    return in_maps, runs, tile_base, T


def prepare(x, edge_src, edge_dst, W, b):
    """Build (nc, in_maps) for the 8-core SPMD run."""
    n_nodes = int(np.asarray(x).shape[0])
    NS = n_nodes // N_CORES
    qn = min(QN, n_nodes)
    in_maps, runs, tile_base, T = _make_in_maps(
        x, edge_src, edge_dst, W, b, n_nodes, N_CORES, qn
    )
    nc = _build_program(n_nodes, NS, runs, tile_base, T, qn)
    return nc, in_maps


def kernel(x, edge_src, edge_dst, W, b):
    global last_results
    n_nodes = int(np.asarray(x).shape[0])
    NS = n_nodes // N_CORES

    nc, in_maps = prepare(x, edge_src, edge_dst, W, b)
    res = run_bass_kernel_spmd(nc, in_maps, core_ids=list(range(N_CORES)))
    last_results = res

    out = np.empty((n_nodes, D), np.float32)
    for c in range(N_CORES):
        out[c * NS : (c + 1) * NS] = res.results[c]["outT"].T
    return out



# revision 10
# speedup vs baseline: 2.1122x; 2.1122x over previous
"""PosGCN Trainium2 kernel: out = x + relu(segment_sum((x@W)[edge_src], edge_dst) + b).

Distribution: 1D node partition across 8 NeuronCores. Core c owns dst nodes
[c*12500, (c+1)*12500) and the edges incident to them (partitioned by dst).
W/b replicated; x replicated into every core's HBM so cross-partition source
rows are gathered locally (no collectives).

Key algebraic rewrite: aggregation is linear, so
    segment_sum((x@W)[src]) == segment_sum(x[src]) @ W
We aggregate raw x features per dst first (the memory-bound part), then apply
the dense transform to the 12500 aggregated rows per core.

v2 design (PE-stream bound previously, ~3x faster than v1):
- x gathered in bf16 only (256B rows; ~1e-3 rel err, gate is 2e-2). ONE
  matmul per 128-edge tile: lhsT = gathered tile (stationary), rhs = one-hot.
- One-hot generated in ONE DVE op per dst-chunk ([128, ntk, 128] bf16
  is_equal of broadcast dstr vs broadcast iota) instead of per tile.
- Gathers batched: chunks processed in groups of GK; one dma_gather per
  (group, quadrant) (~2.5k indices each) round-robined over 4 SWDGE queues.
- PSUM->SBUF copy and relu+bias on the (otherwise idle) scalar engine.

Gather: InstDMAGatherAnt indices are int16, so x rows are addressed
quadrant-relative (4 quadrants of 25000 rows); each core's edges are sorted
by (dst-chunk, src-quadrant) and each (chunk, quadrant) run is padded to a
multiple of 128 edges (pad: src=quadrant row 0, dst sentinel 128 -> one-hot
column of zeros -> contributes nothing).
"""

import sys
from contextlib import ExitStack

import numpy as np

sys.path.insert(0, "/opt/trn_rl_repo")

import ml_dtypes

import concourse.bass as bass
import concourse.tile as tile
from concourse import bacc, mybir
from concourse.bass_utils import run_bass_kernel_spmd

P = 128
N_NODES = 100000
N_EDGES = 1600000
D = 128
N_CORES = 8
QN = 25000  # quadrant size (int16 gather indices must stay < 32768)
GK = 4  # chunks per gather group

f32 = mybir.dt.float32
bf16 = mybir.dt.bfloat16
i16 = mybir.dt.int16

# test.py can read results metadata from here after a run
last_results = None


def _cdiv(a, b):
    return (a + b - 1) // b


def _build_tables(edge_src, edge_dst, n_nodes, n_cores, qn):
    """Sort each core's edges by (dst-chunk, src-quadrant); build tables.

    Two tile orderings over the same edge set:
      gather order: for group g (GK chunks), for quadrant q, for chunk k in g
        -> idx16 table; one dma_gather per (g, q) section (contiguous).
      chunk order: for chunk k, for quadrant q -> dstr table; one one-hot DVE
        op per chunk (contiguous columns).
    The j-th edge of a (k, q) run is at local tile j//128 lane j%128 in both.

    Returns (meta, per_core) where meta describes the SPMD-identical program
    structure and per_core[c] = (idx16_tbl [128, T*8] int16,
    dstr_tbl [128, T] bf16).
    """
    NS = n_nodes // n_cores
    CH = _cdiv(NS, P)
    nq = _cdiv(n_nodes, qn)

    order = np.argsort(edge_dst, kind="stable")
    ss = np.asarray(edge_src)[order].astype(np.int64)
    ds = np.asarray(edge_dst)[order].astype(np.int64)
    bounds = np.searchsorted(ds, np.arange(n_cores + 1) * NS)

    counts = np.zeros((n_cores, CH, nq), np.int64)
    segs = []
    for c in range(n_cores):
        b0, b1 = bounds[c], bounds[c + 1]
        src_c = ss[b0:b1]
        ldst = ds[b0:b1] - c * NS
        quad = src_c // qn
        o2 = np.lexsort((quad, ldst // P))
        src_c, ldst, quad = src_c[o2], ldst[o2], quad[o2]
        np.add.at(counts[c], (ldst // P, quad), 1)
        segs.append((src_c, ldst))

    tiles_kq = _cdiv(counts.max(axis=0), P)  # [CH, nq]
    for k in range(CH):
        if tiles_kq[k].sum() == 0:
            tiles_kq[k][0] = 1  # keep >=1 tile so the chunk's PSUM is written

    ntk = tiles_kq.sum(axis=1)  # tiles per chunk

    groups = [list(range(g0, min(g0 + GK, CH))) for g0 in range(0, CH, GK)]
    # gather order
    goff = {}  # (k, q) -> offset of the tile block within its group buffer
    gsec = []  # per group: list of (q, sec_off, sec_nt)
    gbase = []  # per group: global gather-tile base
    ntg = []  # tiles per group
    t = 0
    for gks in groups:
        gbase.append(t)
        off = 0
        secs = []
        for q in range(nq):
            s0 = off
            for k in gks:
                goff[(k, q)] = off
                off += int(tiles_kq[k][q])
            if off > s0:
                secs.append((q, s0, off - s0))
        gsec.append(secs)
        ntg.append(off)
        t += off
    T = t

    # chunk order
    ctbase = np.zeros(CH + 1, np.int64)
    for k in range(CH):
        ctbase[k + 1] = ctbase[k] + ntk[k]
    assert ctbase[CH] == T
    # chunk tile j -> (q, jq)
    chunk_qj = []
    for k in range(CH):
        qj = []
        for q in range(nq):
            for jq in range(int(tiles_kq[k][q])):
                qj.append((q, jq))
        chunk_qj.append(qj)

    meta = dict(
        CH=CH, nq=nq, groups=groups, goff=goff, gsec=gsec, gbase=gbase,
        ntg=ntg, ntk=ntk, ctbase=ctbase, chunk_qj=chunk_qj,
        tiles_kq=tiles_kq, T=T,
    )

    per_core = []
    for c in range(n_cores):
        src_c, ldst = segs[c]
        # per-(k,q) offsets into the sorted edge arrays
        offs = np.zeros((CH, nq + 1), np.int64)
        offs[:, 1:] = np.cumsum(counts[c], axis=1)
        row_off = np.concatenate([[0], np.cumsum(counts[c].sum(axis=1))])
        flat_src = np.zeros(T * P, np.int16)
        flat_dstr = np.full(T * P, float(P), np.float32)
        for gi, gks in enumerate(groups):
            for k in gks:
                for q in range(nq):
                    n = int(counts[c][k][q])
                    nt = int(tiles_kq[k][q])
                    if nt == 0:
                        continue
                    s = int(row_off[k] + offs[k][q])
                    gslot = (gbase[gi] + goff[(k, q)]) * P
                    if n:
                        flat_src[gslot : gslot + n] = (
                            src_c[s : s + n] - q * qn
                        ).astype(np.int16)
                    cslot = (
                        int(ctbase[k])
                        + sum(int(tiles_kq[k][qq]) for qq in range(q))
                    ) * P
                    if n:
                        flat_dstr[cslot : cslot + n] = (
                            ldst[s : s + n] % P
                        ).astype(np.float32)
        # dma_gather wrapped-index layout: idxs[p, s] = flat[s*16 + p], p<16,
        # replicated over the other 112 partitions
        idx16 = np.tile(flat_src.reshape(T * 8, 16).T, (8, 1))
        dstr_bf = flat_dstr.reshape(T, P).T.astype(ml_dtypes.bfloat16)
        per_core.append(
            (np.ascontiguousarray(idx16), np.ascontiguousarray(dstr_bf))
        )
    return meta, per_core


def _build_program(n_nodes, NS, meta, qn, reps=1, skip=()):
    """Emit the SPMD Bass program for one core (identical across cores).

    reps > 1 replicates the whole body (for slope-based HW timing).
    """
    CH = meta["CH"]
    T = meta["T"]
    groups = meta["groups"]
    last_w = NS - (CH - 1) * P
    ntg_max = int(max(meta["ntg"]))
    ntk_max = int(max(meta["ntk"]))

    nc = bacc.Bacc(
        "TRN2",
        target_bir_lowering=False,
        debug=False,
        num_devices=N_CORES,
        num_swdge_queues=4,
    )

    # Rows padded to 512B (x bf16 in cols 0:D, zeros after): the SWDGE gather
    # locks up the device with a 256B row stride; 512B stride + 256B payload
    # is the proven-fast configuration.
    xp = nc.dram_tensor("xp", [n_nodes, 2 * D], bf16, kind="ExternalInput")
    idx16 = nc.dram_tensor("idx16", [P, T * 8], i16, kind="ExternalInput")
    dstr = nc.dram_tensor("dstr", [P, T], bf16, kind="ExternalInput")
    xrt = nc.dram_tensor("xrt", [P, NS], f32, kind="ExternalInput")
    w = nc.dram_tensor("w", [P, D], f32, kind="ExternalInput")
    bcol = nc.dram_tensor("b", [P, 1], f32, kind="ExternalInput")
    # iota replicated ntk_max times so the one-hot DVE op's in1 has normal
    # strides (middle-dim stride-0 inputs are not HW-safe on DVE)
    iota = nc.dram_tensor("iota", [P, ntk_max * P], bf16, kind="ExternalInput")
    outT = nc.dram_tensor("outT", [P, NS], f32, kind="ExternalOutput")

    with tile.TileContext(nc) as tc, ExitStack() as ctx:
        const = ctx.enter_context(tc.tile_pool(name="const", bufs=1))
        tbl = ctx.enter_context(tc.tile_pool(name="tbl", bufs=1))
        gpool = ctx.enter_context(tc.tile_pool(name="gather", bufs=3))
        opool = ctx.enter_context(tc.tile_pool(name="onehot", bufs=3))
        apsum = ctx.enter_context(tc.tile_pool(name="apsum", bufs=4, space="PSUM"))
        opsum = ctx.enter_context(tc.tile_pool(name="opsum", bufs=2, space="PSUM"))
        asb = ctx.enter_context(tc.tile_pool(name="asb", bufs=2))
        osb = ctx.enter_context(tc.tile_pool(name="osb", bufs=3))
        xrp = ctx.enter_context(tc.tile_pool(name="xrp", bufs=3))

        def emit_body():
            w_sb = const.tile([P, D], f32)
            nc.sync.dma_start(out=w_sb[:], in_=w[:])
            b_sb = const.tile([P, 1], f32)
            nc.sync.dma_start(out=b_sb[:], in_=bcol[:])
            io_sb = const.tile([P, ntk_max, P], bf16)
            nc.sync.dma_start(out=io_sb[:], in_=iota[:])
            idx_sb = tbl.tile([P, T * 8], i16)
            nc.sync.dma_start(out=idx_sb[:], in_=idx16[:])
            dstr_sb = tbl.tile([P, T], bf16)
            nc.sync.dma_start(out=dstr_sb[:], in_=dstr[:])

            if "gather" in skip:
                dummy = const.tile([P, 1, D], bf16, tag="dummy")
                nc.vector.memset(dummy[:], 0.0)
            if "onehot" in skip:
                ohc = const.tile([P, ntk_max, P], bf16, tag="ohc")
                nc.vector.memset(ohc[:], 0.0)

            qctr = 0
            for gi, gks in enumerate(groups):
                if "gather" not in skip:
                    gb = gpool.tile([P, ntg_max, D], bf16, tag="gb")
                    for q, s0, snt in meta["gsec"][gi]:
                        # hard HW limit: 1024 indices (8 tiles) per dma_gather
                        for p0 in range(0, snt, 8):
                            pnt = min(8, snt - p0)
                            gt0 = meta["gbase"][gi] + s0 + p0
                            nc.gpsimd.dma_gather(
                                gb[:, s0 + p0 : s0 + p0 + pnt, :],
                                xp[q * qn : (q + 1) * qn, 0:D],
                                idx_sb[:, gt0 * 8 : (gt0 + pnt) * 8],
                                pnt * P,
                                pnt * P,
                                D,
                                elem_step=2 * D,
                                queue_num=qctr % 4,
                            )
                            qctr += 1
                for k in gks:
                    ntk_ = int(meta["ntk"][k])
                    ct0 = int(meta["ctbase"][k])
                    if "onehot" in skip:
                        oh = ohc
                    else:
                        oh = opool.tile([P, ntk_max, P], bf16, tag="oh")
                        nc.vector.tensor_tensor(
                            out=oh[:, :ntk_, :],
                            in0=dstr_sb[:, ct0 : ct0 + ntk_].to_broadcast(
                                [P, ntk_, P]
                            ),
                            in1=io_sb[:, :ntk_, :],
                            op=mybir.AluOpType.is_equal,
                        )
                    psum = apsum.tile([P, P], f32)
                    if "mm" in skip:
                        nc.vector.memset(psum[:], 0.0)
                    else:
                        for j in range(ntk_):
                            q, jq = meta["chunk_qj"][k][j]
                            gpos = meta["goff"][(k, q)] + jq
                            lhs = (
                                dummy[:, 0, :]
                                if "gather" in skip
                                else gb[:, gpos, :]
                            )
                            nc.tensor.matmul(
                                out=psum[:],
                                lhsT=lhs,
                                rhs=oh[:, j, :],
                                start=(j == 0),
                                stop=(j == ntk_ - 1),
                            )

                    # aggT[f, d] (PSUM) -> out2T[g, d] = sum_f W[f,g]*aggT[f,d]
                    aggT = asb.tile([P, P], f32)
                    nc.scalar.activation(
                        out=aggT[:],
                        in_=psum[:],
                        func=mybir.ActivationFunctionType.Copy,
                    )
                    po = opsum.tile([P, P], f32)
                    nc.tensor.matmul(
                        out=po[:], lhsT=w_sb[:], rhs=aggT[:], start=True, stop=True
                    )
                    ot = osb.tile([P, P], f32)
                    nc.scalar.activation(
                        out=ot[:],
                        in_=po[:],
                        func=mybir.ActivationFunctionType.Relu,
                        bias=b_sb[:],
                    )
                    wk = P if k < CH - 1 else last_w
                    xr = xrp.tile([P, P], f32)
                    nc.sync.dma_start(out=xr[:, :wk], in_=xrt[:, k * P : k * P + wk])
                    nc.vector.tensor_add(out=ot[:, :wk], in0=ot[:, :wk], in1=xr[:, :wk])
                    nc.sync.dma_start(out=outT[:, k * P : k * P + wk], in_=ot[:, :wk])

        for _rep in range(reps):
            emit_body()

    nc.compile()
    return nc


def _make_in_maps(x, edge_src, edge_dst, W, b, n_nodes, n_cores, qn):
    NS = n_nodes // n_cores
    xf = np.ascontiguousarray(np.asarray(x, dtype=np.float32))
    x_bf = xf.astype(ml_dtypes.bfloat16)
    xp = np.zeros((x_bf.shape[0], 2 * D), ml_dtypes.bfloat16)
    xp[:, :D] = x_bf

    meta, per_core = _build_tables(
        np.asarray(edge_src), np.asarray(edge_dst), n_nodes, n_cores, qn
    )

    w_np = np.ascontiguousarray(np.asarray(W, dtype=np.float32))
    b_np = np.ascontiguousarray(np.asarray(b, dtype=np.float32).reshape(P, 1))
    ntk_max = int(max(meta["ntk"]))
    iota_np = np.ascontiguousarray(
        np.broadcast_to(
            np.tile(np.arange(P, dtype=np.float32), ntk_max).astype(
                ml_dtypes.bfloat16
            ),
            (P, ntk_max * P),
        )
    )

    in_maps = []
    for c in range(n_cores):
        idx16_tbl, dstr_tbl = per_core[c]
        in_maps.append(
            {
                "xp": xp,
                "idx16": idx16_tbl,
                "dstr": dstr_tbl,
                "xrt": np.ascontiguousarray(xf[c * NS : (c + 1) * NS].T),
                "w": w_np,
                "b": b_np,
                "iota": iota_np,
            }
        )
    return in_maps, meta


def prepare(x, edge_src, edge_dst, W, b):
    """Build (nc, in_maps) for the 8-core SPMD run."""
    n_nodes = int(np.asarray(x).shape[0])
    NS = n_nodes // N_CORES
    qn = min(QN, n_nodes)
    in_maps, meta = _make_in_maps(
        x, edge_src, edge_dst, W, b, n_nodes, N_CORES, qn
    )
    nc = _build_program(n_nodes, NS, meta, qn)
    return nc, in_maps


def kernel(x, edge_src, edge_dst, W, b):
    global last_results
    n_nodes = int(np.asarray(x).shape[0])
    NS = n_nodes // N_CORES

    nc, in_maps = prepare(x, edge_src, edge_dst, W, b)
    res = run_bass_kernel_spmd(nc, in_maps, core_ids=list(range(N_CORES)))
    last_results = res

    out = np.empty((n_nodes, D), np.float32)
    for c in range(N_CORES):
        out[c * NS : (c + 1) * NS] = res.results[c]["outT"].T
    return out
